# revision 1
# baseline (speedup 1.0000x reference)
"""Trainium2 Bass kernel for nn_CustomRGCN (2-layer RGCN + BN + MLP).

Strategy (8 NeuronCores, SPMD, one NEFF):
- Nodes sharded by contiguous range: core c owns nodes [c*6250, (c+1)*6250),
  padded to 6272 = 49*128 rows per shard.
- All dense transforms run channel-major ([ch, nodes]) with fp32r matmuls
  (weights as lhsT, activations as rhs, node tiles of 512).
- RGCN aggregation = aggregate-then-transform: per (dst-node-tile, relation)
  block, gather raw source features x[src] (128 edges/tile, indirect DMA from
  the AllGather'ed node-major table) and matmul with a host-built scaled
  one-hot matrix S (S[e, slot] = 1/cnt(dst,rel)) accumulating segment MEANS
  in PSUM (node-major). Means are PE-transposed to channel-major and hit
  with the 3 relation weights + root weight (fp32r, N<=512).
- b_rgcn is dropped: training-mode BN immediately follows and cancels any
  constant channel shift.
- BN stats: per-channel sum/sumsq reduced along the free (node) axis,
  AllReduce'd across cores (N=50000 biased stats), then y = Lrelu(h*s + t)
  fused on the scalar engine.
- Collectives: AllGather of each layer input's node-major shard, AllReduce
  of [2,320] stats. Edge routing/sorting/padding and S construction happen
  on host in numpy.
"""
import sys
sys.path.insert(0, '/opt/trn_rl_repo')

import numpy as np

N = 50000
E = 500000
D = 320
R = 3
NCORES = 8
NC_NODES = N // NCORES          # 6250
P = 128
NT = (NC_NODES + P - 1) // P    # 49 node tiles per core
NPAD = NT * P                   # 6272
NBLK = NT * R                   # 147 blocks per core
NG = 512
EPS = 1e-5
LRELU_ALPHA = 0.01

_GROUPS = []
_g0 = 0
while _g0 < NPAD:
    _ng = min(NG, NPAD - _g0)
    _real = max(0, min(NC_NODES - _g0, _ng))
    _GROUPS.append((_g0, _ng, _real))
    _g0 += _ng


def _prep_graph(edge_index, edge_type):
    """Route edges to dst shards, sort by (node-tile, rel) block, build
    per-tile gather indices and the scaled one-hot S (transposed layout)."""
    src = np.asarray(edge_index[0], dtype=np.int64)
    dst = np.asarray(edge_index[1], dtype=np.int64)
    rel = np.asarray(edge_type, dtype=np.int64)

    owner = dst // NC_NODES
    dstl = dst - owner * NC_NODES
    src_ag = (src // NC_NODES) * NPAD + (src % NC_NODES)
    bid = (dstl // P) * R + rel
    slot = dstl % P
    seg = dstl * R + rel

    cnts = np.zeros((NCORES, NBLK), np.int64)
    for c in range(NCORES):
        cnts[c] = np.bincount(bid[owner == c], minlength=NBLK)
    tiles_pb = np.maximum(1, -(-cnts.max(axis=0) // P))
    tile_start = np.zeros(NBLK + 1, np.int64)
    np.cumsum(tiles_pb, out=tile_start[1:])
    T = int(tile_start[-1])

    tsrc = np.zeros((NCORES, T * P), np.int32)
    S_T = np.zeros((NCORES, P, T * P), np.float32)   # [P(edge), T*P(tile,slot)]
    icnt_blk = np.ones((NCORES, P, NBLK), np.float32)  # [slot, block] 1/cnt

    for c in range(NCORES):
        m = owner == c
        bid_c, slot_c, src_c, seg_c = bid[m], slot[m], src_ag[m], seg[m]
        segcnt = np.bincount(seg_c, minlength=NC_NODES * R)
        icnt_e = (1.0 / segcnt[seg_c]).astype(np.float32)
        order = np.argsort(bid_c, kind='stable')
        bid_s = bid_c[order]
        cnt_c = cnts[c]
        within = np.arange(len(bid_s)) - np.repeat(
            np.concatenate(([0], np.cumsum(cnt_c)[:-1])), cnt_c)
        pos = tile_start[:-1][bid_s] * P + within
        tsrc[c, pos] = src_c[order]
        # S_T[c, e, t*P + s] where e = pos % P, t = pos // P, s = slot
        e_in_tile = pos % P
        t_of = pos // P
        S_T[c, e_in_tile, t_of * P + slot_c[order]] = 1.0
        icnt_blk[c][:, :] = 1.0 / np.maximum(
            segcnt.reshape(NC_NODES, R)[
                np.minimum(np.arange(NT * P), NC_NODES - 1)].reshape(
                NT, P, R).transpose(1, 0, 2).reshape(P, NBLK), 1.0)

    import ml_dtypes
    tsrc_cols = tsrc.reshape(NCORES, T, P).transpose(0, 2, 1).copy()
    return dict(T=T, tile_start=tile_start,
                S_T=S_T.astype(ml_dtypes.bfloat16),
                icnt_blk=icnt_blk, tsrc_cols=tsrc_cols)


def _shard_T(x, pad_to=NPAD):
    F = x.shape[1]
    out = np.zeros((NCORES, F, pad_to), x.dtype)
    for c in range(NCORES):
        out[c, :, :NC_NODES] = x[c * NC_NODES:(c + 1) * NC_NODES].T
    return out


def _build_nc(T, tile_start, profile=False):
    import os
    _ab = os.environ.get("K_ABLATE", "base") if profile else "base"
    import concourse.bass as bass
    import concourse.tile as tile
    import concourse.bacc as bacc
    import concourse.mybir as mybir
    from concourse.masks import make_identity
    from contextlib import ExitStack

    f32 = mybir.dt.float32
    f32r = mybir.dt.float32r
    bf16 = mybir.dt.bfloat16
    i32 = mybir.dt.int32

    nc = bacc.Bacc("TRN2", target_bir_lowering=False, debug=False,
                   num_devices=1 if profile else NCORES)

    def din(name, shape):
        return nc.dram_tensor(name, shape, f32, kind="ExternalInput")

    ncat_T = din("ncat_T", [8, NPAD])
    tweet_T = din("tweet_T", [768, NPAD])
    user_T = din("user_T", [768, NPAD])
    uname_T = din("uname_T", [768, NPAD])
    w_ncat = din("w_ncat", [8, 128])
    b_c0 = din("b_c0", [128, 1])
    w_tw = din("w_tw", [768, 64])
    w_us = din("w_us", [768, 64])
    w_un = din("w_un", [768, 64])
    b_tw = din("b_tw", [64, 1])
    b_us = din("b_us", [64, 1])
    b_c2 = din("b_c2", [64, 1])
    w_in = din("w_in", [D, D])
    b_in = din("b_in", [D, 1])
    w_rel = [din(f"w_rel{l}", [R, D, D]) for l in (1, 2)]
    w_root = [din(f"w_root{l}", [D, D]) for l in (1, 2)]
    gamma = [din(f"gamma{l}", [D, 1]) for l in (1, 2)]
    beta = [din(f"beta{l}", [D, 1]) for l in (1, 2)]
    w_o1 = din("w_o1", [D, D])
    b_o1 = din("b_o1", [D, 1])
    w_o2 = din("w_o2", [D, D])
    b_o2 = din("b_o2", [D, 1])
    w_o3 = din("w_o3", [D, 2])
    b_o3 = din("b_o3", [2, 1])
    s_matT = nc.dram_tensor("s_matT", [P, T * P], bf16, kind="ExternalInput")
    icnt = din("icnt", [P, NBLK])
    tsrc = nc.dram_tensor("tsrc", [P, T], i32, kind="ExternalInput")

    out = nc.dram_tensor("out", [NC_NODES, 2], f32, kind="ExternalOutput")

    agi = [nc.dram_tensor(f"agi{l}", [NPAD, D], bf16) for l in (1, 2)]
    ago = [nc.dram_tensor(f"ago{l}", [NCORES * NPAD, D], bf16) for l in (1, 2)]
    xcm = [nc.dram_tensor(f"xcm{l}", [D, NPAD], f32) for l in (0, 1, 2)]
    hcm = [nc.dram_tensor(f"hcm{l}", [D, NPAD], f32) for l in (1, 2)]
    arin = [nc.dram_tensor(f"arin{l}", [2, D], f32) for l in (1, 2)]
    arout = [nc.dram_tensor(f"arout{l}", [2, D], f32) for l in (1, 2)]

    KCH = [(0, 128), (128, 128), (256, 64)]
    RG = [list(range(NCORES))]

    with tile.TileContext(nc) as tc:
        ctx = ExitStack()
        wp = ctx.enter_context(tc.tile_pool(name="weights", bufs=1))
        cp = ctx.enter_context(tc.tile_pool(name="consts", bufs=1))
        sbg = ctx.enter_context(tc.tile_pool(name="gather", bufs=1))
        sba = ctx.enter_context(tc.tile_pool(name="acts", bufs=1))
        sbh = ctx.enter_context(tc.tile_pool(name="hout", bufs=1))
        sbt = ctx.enter_context(tc.tile_pool(name="stats", bufs=1))
        psA = ctx.enter_context(tc.tile_pool(name="psA", bufs=2, space="PSUM"))
        psT = ctx.enter_context(tc.tile_pool(name="psT", bufs=3, space="PSUM"))
        psB = ctx.enter_context(tc.tile_pool(name="psB", bufs=3, space="PSUM"))

        ident = cp.tile([P, P], f32)
        make_identity(nc, ident[:])
        eps_sb = cp.tile([P, 1], f32)
        nc.gpsimd.memset(eps_sb[:], EPS)
        idxt = cp.tile([P, T], i32)
        nc.sync.dma_start(out=idxt[:], in_=tsrc[:, :])
        icnt_sb = cp.tile([P, NBLK], f32)
        nc.sync.dma_start(out=icnt_sb[:], in_=icnt[:, :])

        def wload(name, ap, kk, cols, dt=None):
            dt = f32r if dt is None else dt
            t = wp.tile([kk, cols], dt, name=name)
            nc.sync.dma_start(out=t[:], in_=ap.bitcast(f32r) if dt is f32r else ap)
            return t

        w_nc_sb = wload("w_nc_sb", w_ncat[:, :], 8, 128)
        w_tw_sb = [wload(f"w_tw{j}", w_tw[j*128:(j+1)*128, :], 128, 64) for j in range(6)]
        w_us_sb = [wload(f"w_us{j}", w_us[j*128:(j+1)*128, :], 128, 64) for j in range(6)]
        w_un_sb = [wload(f"w_un{j}", w_un[j*128:(j+1)*128, :], 128, 64) for j in range(6)]
        KIN = [(0, 128), (128, 64), (192, 64), (256, 64)]
        w_in_sb = [wload(f"w_in{k}", w_in[k0:k0+kk, :], kk, D) for k, (k0, kk) in enumerate(KIN)]
        w_rel_sb = [[[wload(f"w_rel{l}_{r}_{k}", w_rel[l][r, k0:k0+kk, :], kk, D)
                      for k, (k0, kk) in enumerate(KCH)] for r in range(R)]
                    for l in (0, 1)]
        w_root_sb = [[wload(f"w_root{l}_{k}", w_root[l][k0:k0+kk, :], kk, D)
                      for k, (k0, kk) in enumerate(KCH)] for l in (0, 1)]
        w_o1_sb = [wload(f"w_o1_{k}", w_o1[k0:k0+kk, :], kk, D) for k, (k0, kk) in enumerate(KCH)]
        w_o2_sb = [wload(f"w_o2_{k}", w_o2[k0:k0+kk, :], kk, D) for k, (k0, kk) in enumerate(KCH)]
        w_o3_sb = [wload(f"w_o3_{k}", w_o3[k0:k0+kk, :], kk, 2) for k, (k0, kk) in enumerate(KCH)]

        def bload(name, ap, kk):
            t = cp.tile([kk, 1], f32, name=name)
            nc.sync.dma_start(out=t[:], in_=ap)
            return t

        b_c0_sb = bload("b_c0_sb", b_c0[:, :], 128)
        b_tw_sb = bload("b_tw_sb", b_tw[:, :], 64)
        b_us_sb = bload("b_us_sb", b_us[:, :], 64)
        b_c2_sb = bload("b_c2_sb", b_c2[:, :], 64)
        b_in_sb = [bload(f"b_in{k}", b_in[k0:k0+kk, :], kk) for k, (k0, kk) in enumerate(KCH)]
        b_o1_sb = [bload(f"b_o1_{k}", b_o1[k0:k0+kk, :], kk) for k, (k0, kk) in enumerate(KCH)]
        b_o2_sb = [bload(f"b_o2_{k}", b_o2[k0:k0+kk, :], kk) for k, (k0, kk) in enumerate(KCH)]
        b_o3_sb = bload("b_o3_sb", b_o3[:, :], 2)
        gamma_sb = [[bload(f"gam{l}_{k}", gamma[l][k0:k0+kk, :], kk)
                     for k, (k0, kk) in enumerate(KCH)] for l in (0, 1)]
        beta_sb = [[bload(f"bet{l}_{k}", beta[l][k0:k0+kk, :], kk)
                    for k, (k0, kk) in enumerate(KCH)] for l in (0, 1)]

        Lrelu = mybir.ActivationFunctionType.Lrelu
        Square = mybir.ActivationFunctionType.Square
        Sqrt = mybir.ActivationFunctionType.Sqrt
        Ident = mybir.ActivationFunctionType.Identity
        AX = mybir.AxisListType.X

        def tr_nm2cm(src_ap, mm):
            """[128(nodes), mm(ch)] -> psum [mm(ch), 128(nodes)]; K=128."""
            pt = psT.tile([P, P], f32, tag="pt")
            nc.tensor.transpose(out=pt[:mm, :], in_=src_ap.bitcast(f32),
                                identity=ident[:, :])
            return pt

        def tr_cm2nm(src_ap, mm):
            """[mm(ch), 128(nodes)] -> psum [128(nodes), 128]; K=mm."""
            pt = psT.tile([P, P], f32, tag="pt")
            nc.tensor.transpose(out=pt[:, :], in_=src_ap.bitcast(f32),
                                identity=ident[:mm, :])
            return pt

        # =========================================================
        # Phase 1: input projection -> x1 (cm + nm shard)
        # =========================================================
        for (g0, ng, real) in _GROUPS:
            # x0 pieces: [0]=num+cat (128ch), [1]=tweet, [2]=user, [3]=uname
            x0 = [sba.tile([P, NG], f32r, name=f"x0_{m}", tag=f"a{m % 3}", bufs=4)
                  for m in range(4)]
            pa = psB.tile([P, NG], f32, tag="pb")
            nct = sbg.tile([8, NG], f32r, tag="nct", bufs=2)
            nc.sync.dma_start(out=nct[:, :ng], in_=ncat_T[:, g0:g0+ng].bitcast(f32r))
            nc.tensor.matmul(pa[:, :ng], w_nc_sb[:], nct[:, :ng], start=True, stop=True)
            nc.scalar.activation(x0[0][:, :ng], pa[:, :ng], Lrelu,
                                 bias=b_c0_sb[:, :1], alpha=LRELU_ALPHA)
            for piece, (tab, wsb, bsb) in enumerate(
                    [(tweet_T, w_tw_sb, b_tw_sb), (user_T, w_us_sb, b_us_sb),
                     (uname_T, w_un_sb, b_c2_sb)]):
                pa = psB.tile([P, NG], f32, tag="pb")
                for j in range(6):
                    et = sbg.tile([P, NG], f32r, tag="emb", bufs=6)
                    nc.sync.dma_start(out=et[:, :ng],
                                      in_=tab[j*128:(j+1)*128, g0:g0+ng].bitcast(f32r))
                    nc.tensor.matmul(pa[0:64, :ng], wsb[j][:], et[:, :ng],
                                     start=(j == 0), stop=(j == 5))
                nc.scalar.activation(x0[1 + piece][0:64, :ng], pa[0:64, :ng],
                                     Lrelu, bias=bsb[:, :1], alpha=LRELU_ALPHA)
            x1 = [sba.tile([P, NG], f32r, name=f"x1_{m}", tag=f"b{m}", bufs=2)
                  for m in range(3)]
            for m, (m0, mm) in enumerate(KCH):
                pb = psB.tile([P, NG], f32, tag="pb")
                for k, (k0, kk) in enumerate(KIN):
                    nc.tensor.matmul(pb[:mm, :ng], w_in_sb[k][:kk, m0:m0+mm],
                                     x0[k][:kk, :ng], start=(k == 0), stop=(k == 3))
                nc.scalar.activation(x1[m][:mm, :ng], pb[:mm, :ng], Lrelu,
                                     bias=b_in_sb[m][:, :1], alpha=LRELU_ALPHA)
                nc.sync.dma_start(out=xcm[0][m0:m0+mm, g0:g0+ng].bitcast(f32r),
                                  in_=x1[m][:mm, :ng])
            for toff in range(0, ng, P):
                t_nm = sbh.tile([P, D], bf16, tag="nm", bufs=4)
                for m, (m0, mm) in enumerate(KCH):
                    pt = tr_cm2nm(x1[m][:mm, toff:toff+P], mm)
                    nc.vector.tensor_copy(t_nm[:, m0:m0+mm], pt[:, :mm])
                nc.sync.dma_start(out=agi[0][g0+toff:g0+toff+P, :], in_=t_nm[:])

        if profile:
            nc.sync.dma_start(out=ago[0][0:NPAD, :], in_=agi[0][:, :])
        else:
            nc.gpsimd.collective_compute(
                "AllGather", mybir.AluOpType.bypass, replica_groups=RG,
                ins=[agi[0].ap().opt()], outs=[ago[0].ap().opt()])

        # =========================================================
        # RGCN layers
        # =========================================================
        _nlayers = {"p1": 0, "l1": 1}.get(_ab, 2) if profile else 2
        for l in range(_nlayers):
            src_tab = ago[l]
            x_prev = xcm[l]
            sum_st = [sbt.tile([P, 16], f32, name=f"sum{l}_{m}") for m in range(3)]
            sq_st = [sbt.tile([P, 16], f32, name=f"sq{l}_{m}") for m in range(3)]
            for m in range(3):
                nc.vector.memset(sum_st[m][:], 0.0)
                nc.vector.memset(sq_st[m][:], 0.0)

            for gi, (g0, ng, real) in enumerate(_GROUPS):
                t_lo, t_hi = g0 // P, (g0 + ng) // P
                mcm = [[sba.tile([P, NG], f32r, name=f"mcm_{r}_{k}",
                                 tag=f"mcm_{r}_{k}", bufs=2)
                        for k in range(3)] for r in range(R)]
                for t in range(t_lo, t_hi):
                    toff = (t - t_lo) * P
                    for r in range(R):
                        b = t * R + r
                        i0, i1 = int(tile_start[b]), int(tile_start[b + 1])
                        pm = psA.tile([P, D], f32, tag="pm")
                        for ci in range(i0, i1, 4):
                            cn = min(4, i1 - ci)
                            st = sbg.tile([P, 4 * P], bf16, tag="st", bufs=6)
                            nc.sync.dma_start(
                                out=st[:, :cn * P],
                                in_=s_matT[:, ci*P:(ci+cn)*P])
                            for i in range(ci, ci + cn):
                                gt = sbg.tile([P, D], bf16, tag="gt", bufs=16)
                                if _ab != "nogather":
                                    nc.gpsimd.indirect_dma_start(
                                        out=gt[:], out_offset=None,
                                        in_=src_tab[:, :],
                                        in_offset=bass.IndirectOffsetOnAxis(
                                            ap=idxt[:, i:i+1], axis=0))
                                else:
                                    nc.sync.dma_start(
                                        out=gt[:],
                                        in_=src_tab[(i % 64)*P:(i % 64+1)*P, :])
                                if _ab != "nosegmm":
                                    nc.tensor.matmul(
                                        pm[:, :], st[:, (i-ci)*P:(i-ci+1)*P], gt[:],
                                        start=(i == i0), stop=(i == i1 - 1))
                                elif i == i0:
                                    nc.tensor.matmul(
                                        pm[:, :], st[:, 0:P], gt[:],
                                        start=True, stop=True)
                        mnm = sbh.tile([P, D], f32r, tag="mnm", bufs=6)
                        if _ab != "noscale":
                            nc.vector.tensor_scalar_mul(mnm[:], pm[:],
                                                        icnt_sb[:, b:b+1])
                        if _ab not in ("notrans", "noscale"):
                            for m, (m0, mm) in enumerate(KCH):
                                pt = tr_nm2cm(mnm[:, m0:m0+mm], mm)
                                nc.vector.tensor_copy(
                                    mcm[r][m][:mm, toff:toff+P], pt[:mm, :])

                for m, (m0, mm) in enumerate(KCH):
                    pb = psB.tile([P, NG], f32, tag="pb")
                    first = True
                    if _ab != "nodense":
                        for r in range(R):
                            for k, (k0, kk) in enumerate(KCH):
                                nc.tensor.matmul(pb[:mm, :ng],
                                                 w_rel_sb[l][r][k][:kk, m0:m0+mm],
                                                 mcm[r][k][:kk, :ng],
                                                 start=first, stop=False)
                                first = False
                    for k, (k0, kk) in enumerate(KCH):
                        xt = sbg.tile([P, NG], f32r, tag="xt", bufs=3)
                        nc.sync.dma_start(out=xt[:kk, :ng],
                                          in_=x_prev[k0:k0+kk, g0:g0+ng].bitcast(f32r))
                        nc.tensor.matmul(pb[:mm, :ng],
                                         w_root_sb[l][k][:kk, m0:m0+mm],
                                         xt[:kk, :ng], start=first and k == 0,
                                         stop=(k == 2))
                    hsb = sbh.tile([P, NG], f32, tag="hsb", bufs=3)
                    nc.vector.tensor_copy(hsb[:mm, :ng], pb[:mm, :ng])
                    nc.sync.dma_start(out=hcm[l][m0:m0+mm, g0:g0+ng],
                                      in_=hsb[:mm, :ng])
                    if _ab != "nostats":
                        nc.vector.reduce_sum(sum_st[m][:mm, gi:gi+1],
                                             hsb[:mm, :real], axis=AX)
                        sq = sbh.tile([P, NG], f32, tag="sq", bufs=2)
                        nc.scalar.activation(sq[:mm, :real], hsb[:mm, :real], Square)
                        nc.vector.reduce_sum(sq_st[m][:mm, gi:gi+1],
                                             sq[:mm, :real], axis=AX)

            # ---- BN stats AllReduce ----
            for m, (m0, mm) in enumerate(KCH):
                s1 = sbt.tile([P, 1], f32, name=f"s1_{l}_{m}")
                s2 = sbt.tile([P, 1], f32, name=f"s2_{l}_{m}")
                nc.vector.reduce_sum(s1[:mm, :], sum_st[m][:mm, :], axis=AX)
                nc.vector.reduce_sum(s2[:mm, :], sq_st[m][:mm, :], axis=AX)
                nc.sync.dma_start(out=arin[l][0:1, m0:m0+mm], in_=s1[:mm, :])
                nc.sync.dma_start(out=arin[l][1:2, m0:m0+mm], in_=s2[:mm, :])
            if profile:
                nc.sync.dma_start(out=arout[l][:, :], in_=arin[l][:, :])
            else:
                nc.gpsimd.collective_compute(
                    "AllReduce", mybir.AluOpType.add, replica_groups=RG,
                    ins=[arin[l].ap().opt()], outs=[arout[l].ap().opt()])

            scl, sft = [], []
            for m, (m0, mm) in enumerate(KCH):
                sg = sbt.tile([P, 1], f32, name=f"sg_{l}_{m}")
                sqg = sbt.tile([P, 1], f32, name=f"sqg_{l}_{m}")
                nc.sync.dma_start(out=sg[:mm, :], in_=arout[l][0:1, m0:m0+mm])
                nc.sync.dma_start(out=sqg[:mm, :], in_=arout[l][1:2, m0:m0+mm])
                mean = sbt.tile([P, 1], f32, name=f"mean_{l}_{m}")
                nc.vector.tensor_scalar_mul(mean[:mm, :], sg[:mm, :], 1.0 / N)
                msq = sbt.tile([P, 1], f32, name=f"msq_{l}_{m}")
                nc.vector.tensor_scalar_mul(msq[:mm, :], sqg[:mm, :], 1.0 / N)
                m2 = sbt.tile([P, 1], f32, name=f"m2_{l}_{m}")
                nc.vector.tensor_tensor(out=m2[:mm, :], in0=mean[:mm, :],
                                        in1=mean[:mm, :], op=mybir.AluOpType.mult)
                var = sbt.tile([P, 1], f32, name=f"var_{l}_{m}")
                nc.vector.tensor_tensor(out=var[:mm, :], in0=msq[:mm, :],
                                        in1=m2[:mm, :], op=mybir.AluOpType.subtract)
                nc.vector.tensor_tensor(out=var[:mm, :], in0=var[:mm, :],
                                        in1=eps_sb[:mm, :], op=mybir.AluOpType.add)
                std = sbt.tile([P, 1], f32, name=f"std_{l}_{m}")
                nc.scalar.activation(std[:mm, :], var[:mm, :], Sqrt)
                istd = sbt.tile([P, 1], f32, name=f"istd_{l}_{m}")
                nc.vector.reciprocal(istd[:mm, :], std[:mm, :])
                sc = sbt.tile([P, 1], f32, name=f"sc_{l}_{m}")
                nc.vector.tensor_tensor(out=sc[:mm, :], in0=gamma_sb[l][m][:mm, :],
                                        in1=istd[:mm, :], op=mybir.AluOpType.mult)
                tmp = sbt.tile([P, 1], f32, name=f"tmp_{l}_{m}")
                nc.vector.tensor_tensor(out=tmp[:mm, :], in0=mean[:mm, :],
                                        in1=sc[:mm, :], op=mybir.AluOpType.mult)
                sh = sbt.tile([P, 1], f32, name=f"sh_{l}_{m}")
                nc.vector.tensor_tensor(out=sh[:mm, :], in0=beta_sb[l][m][:mm, :],
                                        in1=tmp[:mm, :], op=mybir.AluOpType.subtract)
                scl.append(sc)
                sft.append(sh)

            # ---- normalize + lrelu -> y ----
            for (g0, ng, real) in _GROUPS:
                yt = [sba.tile([P, NG], f32r, name=f"yt_{m}", tag=f"a{m}", bufs=4)
                      for m in range(3)]
                for m, (m0, mm) in enumerate(KCH):
                    ht = sbh.tile([P, NG], f32, tag="hsb", bufs=3)
                    nc.sync.dma_start(out=ht[:mm, :ng],
                                      in_=hcm[l][m0:m0+mm, g0:g0+ng])
                    nc.scalar.activation(yt[m][:mm, :ng], ht[:mm, :ng], Lrelu,
                                         bias=sft[m][:mm, :1],
                                         scale=scl[m][:mm, :1],
                                         alpha=LRELU_ALPHA)
                    nc.sync.dma_start(out=xcm[l+1][m0:m0+mm, g0:g0+ng].bitcast(f32r),
                                      in_=yt[m][:mm, :ng])
                if l == 0:
                    for toff in range(0, ng, P):
                        t_nm = sbh.tile([P, D], bf16, tag="nm", bufs=4)
                        for m, (m0, mm) in enumerate(KCH):
                            pt = tr_cm2nm(yt[m][:mm, toff:toff+P], mm)
                            nc.vector.tensor_copy(t_nm[:, m0:m0+mm], pt[:, :mm])
                        nc.sync.dma_start(out=agi[1][g0+toff:g0+toff+P, :],
                                          in_=t_nm[:])
            if l == 0:
                if profile:
                    nc.sync.dma_start(out=ago[1][0:NPAD, :], in_=agi[1][:, :])
                else:
                    nc.gpsimd.collective_compute(
                        "AllGather", mybir.AluOpType.bypass, replica_groups=RG,
                        ins=[agi[1].ap().opt()], outs=[ago[1].ap().opt()])

        # =========================================================
        # MLP head
        # =========================================================
        for (g0, ng, real) in _GROUPS:
            if profile and _ab in ("p1", "l1", "nomlp"):
                break
            if real == 0:
                continue
            z0 = [sba.tile([P, NG], f32r, name=f"z0_{k}", tag=f"a{k}", bufs=4) for k in range(3)]
            for k, (k0, kk) in enumerate(KCH):
                nc.sync.dma_start(out=z0[k][:kk, :ng],
                                  in_=xcm[2][k0:k0+kk, g0:g0+ng].bitcast(f32r))
            z1 = [sba.tile([P, NG], f32r, name=f"z1_{m}", tag=f"b{m}", bufs=2) for m in range(3)]
            for m, (m0, mm) in enumerate(KCH):
                pb = psB.tile([P, NG], f32, tag="pb")
                for k, (k0, kk) in enumerate(KCH):
                    nc.tensor.matmul(pb[:mm, :ng], w_o1_sb[k][:kk, m0:m0+mm],
                                     z0[k][:kk, :ng], start=(k == 0), stop=(k == 2))
                nc.scalar.activation(z1[m][:mm, :ng], pb[:mm, :ng], Lrelu,
                                     bias=b_o1_sb[m][:, :1], alpha=LRELU_ALPHA)
            z2 = [sba.tile([P, NG], f32r, name=f"z2_{m}", tag=f"mcm_0_{m}", bufs=2) for m in range(3)]
            for m, (m0, mm) in enumerate(KCH):
                pb = psB.tile([P, NG], f32, tag="pb")
                for k, (k0, kk) in enumerate(KCH):
                    nc.tensor.matmul(pb[:mm, :ng], w_o2_sb[k][:kk, m0:m0+mm],
                                     z1[k][:kk, :ng], start=(k == 0), stop=(k == 2))
                nc.scalar.activation(z2[m][:mm, :ng], pb[:mm, :ng], Lrelu,
                                     bias=b_o2_sb[m][:, :1], alpha=LRELU_ALPHA)
            po = psB.tile([P, NG], f32, tag="pb")
            for k, (k0, kk) in enumerate(KCH):
                nc.tensor.matmul(po[:2, :ng], w_o3_sb[k][:kk, :],
                                 z2[k][:kk, :ng], start=(k == 0), stop=(k == 2))
            osb = sbh.tile([P, NG], f32, tag="osb", bufs=2)
            nc.scalar.activation(osb[:2, :ng], po[:2, :ng], Ident,
                                 bias=b_o3_sb[:, :1])
            nc.sync.dma_start(out=out[g0:g0+real, :].transpose([1, 0]),
                              in_=osb[:2, :real])

        ctx.close()
    return nc




def _make_runner(nc, n_cores):
    """Compile once; return (prepare, run, unpack) over PJRT/axon shard_map."""
    import jax
    from jax.sharding import Mesh, PartitionSpec, NamedSharding
    from jax.experimental.shard_map import shard_map
    import concourse.mybir as mybir
    from concourse import bass2jax
    from concourse.bass2jax import _bass_exec_p, install_neuronx_cc_hook

    install_neuronx_cc_hook()
    partition_name = nc.partition_id_tensor.name if nc.partition_id_tensor else None

    in_names, out_names, out_avals, zero_outs = [], [], [], []
    for alloc in nc.m.functions[0].allocations:
        if not isinstance(alloc, mybir.MemoryLocationSet):
            continue
        name = alloc.memorylocations[0].name
        if alloc.kind == "ExternalInput":
            if name != partition_name:
                in_names.append(name)
        elif alloc.kind == "ExternalOutput":
            shape = tuple(alloc.tensor_shape)
            dtype = mybir.dt.np(alloc.dtype)
            out_names.append(name)
            out_avals.append(jax.core.ShapedArray(shape, dtype))
            zero_outs.append(np.zeros(shape, dtype))
    n_params = len(in_names)
    n_outs = len(out_avals)
    all_in_names = list(in_names) + list(out_names)
    if partition_name is not None:
        all_in_names.append(partition_name)

    def _body(*args):
        operands = list(args)
        if partition_name is not None:
            operands.append(bass2jax.partition_id_tensor())
        outs = _bass_exec_p.bind(
            *operands,
            out_avals=tuple(out_avals),
            in_names=tuple(all_in_names),
            out_names=tuple(out_names),
            lowering_input_output_aliases=(),
            sim_require_finite=True,
            sim_require_nnan=True,
            nc=nc,
        )
        return tuple(outs)

    devices = jax.devices()[:n_cores]
    mesh = Mesh(np.asarray(devices), ("core",))
    in_specs = (PartitionSpec("core"),) * (n_params + n_outs)
    out_specs = (PartitionSpec("core"),) * len(out_names)
    sharded = jax.jit(
        shard_map(_body, mesh=mesh, in_specs=in_specs, out_specs=out_specs,
                  check_rep=False),
        keep_unused=True,
    )

    def prepare(in_maps):
        sh = NamedSharding(mesh, PartitionSpec("core"))
        concat_in = [
            np.concatenate([np.asarray(in_maps[c][name]) for c in range(n_cores)],
                           axis=0)
            for name in in_names
        ]
        concat_zeros = [
            np.zeros((n_cores * z.shape[0], *z.shape[1:]), z.dtype)
            for z in zero_outs
        ]
        args = [jax.device_put(a, sh) for a in concat_in + concat_zeros]
        for a in args:
            a.block_until_ready()
        return args

    def run(args):
        return sharded(*args)

    def unpack(outs):
        return [
            {name: np.asarray(outs[i]).reshape(n_cores, *out_avals[i].shape)[c]
             for i, name in enumerate(out_names)}
            for c in range(n_cores)
        ]

    return prepare, run, unpack


_CACHE = {}
_LAST_ARGS = None


def kernel(**inputs):
    global _LAST_ARGS
    import jax
    inp = {k: np.asarray(v) for k, v in inputs.items()}

    g = _prep_graph(inp['edge_index'], inp['edge_type'])
    T = g['T']

    if "k" not in _CACHE:
        nc = _build_nc(T, g['tile_start'])
        nc.compile()
        _CACHE["k"] = (_make_runner(nc, NCORES), T)
    (prepare, run, unpack), T_built = _CACHE["k"]
    assert T_built == T, "edge distribution changed between calls"

    f32 = np.float32
    ncat = np.concatenate([inp['num_prop'], inp['cat_prop']], axis=1).astype(f32)
    ncat_T = _shard_T(ncat)
    tweet_T = _shard_T(np.asarray(inp['tweet_emb'], f32))
    user_T = _shard_T(np.asarray(inp['user_emb'], f32))
    uname_T = _shard_T(np.asarray(inp['user_name_emb'], f32))

    w_ncat = np.zeros((8, 128), f32)
    w_ncat[0:5, 0:64] = inp['w_num']
    w_ncat[5:8, 64:128] = inp['w_cat']
    b_c0 = np.concatenate([inp['b_num'], inp['b_cat']])[:, None]
    b_tw = inp['b_tweet'][:, None]
    b_us = inp['b_user'][:, None]
    b_c2 = inp['b_uname'][:, None]

    common = dict(
        w_ncat=w_ncat, b_c0=b_c0, b_tw=b_tw, b_us=b_us, b_c2=b_c2,
        w_tw=inp['w_tweet'], w_us=inp['w_user'], w_un=inp['w_uname'],
        w_in=inp['w_in'], b_in=inp['b_in'][:, None],
        w_rel1=inp['w_rel1'], w_root1=inp['w_root1'],
        gamma1=inp['gamma1'][:, None], beta1=inp['beta1'][:, None],
        w_rel2=inp['w_rel2'], w_root2=inp['w_root2'],
        gamma2=inp['gamma2'][:, None], beta2=inp['beta2'][:, None],
        w_o1=inp['w_o1'], b_o1=inp['b_o1'][:, None],
        w_o2=inp['w_o2'], b_o2=inp['b_o2'][:, None],
        w_o3=inp['w_o3'], b_o3=inp['b_o3'][:, None],
    )
    common = {k: np.ascontiguousarray(v, dtype=f32) for k, v in common.items()}

    in_maps = []
    for c in range(NCORES):
        m = dict(common)
        m['ncat_T'] = ncat_T[c]
        m['tweet_T'] = tweet_T[c]
        m['user_T'] = user_T[c]
        m['uname_T'] = uname_T[c]
        m['s_matT'] = g['S_T'][c]
        m['icnt'] = g['icnt_blk'][c]
        m['tsrc'] = g['tsrc_cols'][c]
        in_maps.append(m)

    args = prepare(in_maps)
    _LAST_ARGS = args
    outs = run(args)
    jax.block_until_ready(outs)
    res = unpack(outs)
    full = np.concatenate([res[c]['out'] for c in range(NCORES)], axis=0)
    return full

